# revision 24
# baseline (speedup 1.0000x reference)
"""Trainium2 Bass kernel for a 3-layer GCN (DeepGRL) on 8 NeuronCores.

Strategy (per the dst-partitioned sharding plan):
  - Nodes are sharded contiguously across the 8 cores; edges are owned by the
    core that owns their destination node.
  - Per layer:  h = a @ W  (dense matmul on PE, per-core own nodes),
    u = dinv * h is written to a DRAM table and AllGather'ed so every core
    holds the full [N, F] table.
  - Aggregation out_i = dinv_i * (sum_{e: dst=i} u[src_e] + u_i) + b is done
    per 128-dst-node block: edge source rows are fetched with the SWDGE
    dma_gather instruction (128 rows per chunk land on 128 partitions), and a
    one-hot "segment matrix" S (built on the vector engine from the dst-local
    ids with an is_equal compare against an iota row) maps edges to dst rows
    via a PE matmul accumulating in PSUM.
  - BatchNorm batch statistics (sum / sum-of-squares per feature) are computed
    with ones-vector matmuls and AllReduce'd across cores; BN apply + ReLU is
    fused into one scalar-engine activation during the transpose back to the
    feature-major layout the next layer's matmul needs.

dma_gather indices are int16, so the gathered table is addressed in two
halves (rows < HALF and rows >= HALF); every dst block's edge list is split
into a "lo" and a "hi" sublist, each padded to a multiple of 128.
"""

import math
from contextlib import ExitStack

import numpy as np

import concourse.bacc as bacc
import concourse.bass as bass
import concourse.mybir as mybir
import concourse.tile as tile
from concourse import library_config
from concourse.bass_utils import run_bass_kernel_spmd

P = 128
F32 = mybir.dt.float32
BF16 = mybir.dt.bfloat16
I16 = mybir.dt.int16
AF = mybir.ActivationFunctionType
ALU = mybir.AluOpType


# ----------------------------------------------------------------------------
# Host-side graph preprocessing
# ----------------------------------------------------------------------------
def make_plan(edge_index, N, n_cores=8, half=32768, gb=4):
    """Partition edges by destination core, build per-core gather index /
    segment-id arrays (compile-time constants of the kernel)."""
    src = np.asarray(edge_index[0], dtype=np.int64)
    dst = np.asarray(edge_index[1], dtype=np.int64)
    E = src.shape[0]

    indeg = np.bincount(dst, minlength=N).astype(np.float64)
    deg = indeg + 1.0  # self loop
    dinv = (1.0 / np.sqrt(deg)).astype(np.float32)

    npc = N // n_cores
    assert npc * n_cores == N
    nblk = math.ceil(npc / P)
    npc_pad = nblk * P

    # source row in the padded global table layout
    src_core = src // npc
    src_row = src_core * npc_pad + (src - src_core * npc)

    dst_core = dst // npc
    dloc = dst - dst_core * npc

    # bucket edges: per (core, block) -> lo list / hi list
    lo_lists = [[[] for _ in range(nblk)] for _ in range(n_cores)]
    hi_lists = [[[] for _ in range(nblk)] for _ in range(n_cores)]
    lo_dl = [[[] for _ in range(nblk)] for _ in range(n_cores)]
    hi_dl = [[[] for _ in range(nblk)] for _ in range(n_cores)]
    order = np.argsort(dst, kind="stable")
    for e in order:
        r = dst_core[e]
        b = dloc[e] // P
        d_in_blk = dloc[e] - b * P
        sr = src_row[e]
        if sr < half:
            lo_lists[r][b].append(sr)
            lo_dl[r][b].append(d_in_blk)
        else:
            hi_lists[r][b].append(sr - half)
            hi_dl[r][b].append(d_in_blk)

    c_lo = max(
        1,
        max(
            math.ceil(len(lo_lists[r][b]) / P)
            for r in range(n_cores)
            for b in range(nblk)
        ),
    )
    n_hi = max(
        len(hi_lists[r][b]) for r in range(n_cores) for b in range(nblk)
    )
    c_hi = math.ceil(n_hi / P)  # may be 0
    cpb = c_lo + c_hi  # chunks per block

    # gather groups of up to `gb` blocks
    groups = []
    b0 = 0
    while b0 < nblk:
        g = min(gb, nblk - b0)
        groups.append((b0, g))
        b0 += g

    def wrap_idx(ids):
        """int16 wrap layout: idx i -> [i % 16, i // 16], replicated to 128
        partitions (8 groups of 16)."""
        ids = np.asarray(ids, dtype=np.int16)
        L = ids.shape[0]
        assert L % 16 == 0
        w = ids.reshape(L // 16, 16).T  # [16, L/16]
        return np.tile(w, (8, 1))  # [128, L/16]

    idx_lo = np.zeros((n_cores, P, nblk * c_lo * 8), dtype=np.int16)
    idx_hi = np.zeros((n_cores, P, max(1, nblk * c_hi * 8)), dtype=np.int16)
    dl_arr = np.full((n_cores, P, nblk * cpb), 300.0, dtype=np.float32)

    for r in range(n_cores):
        lo_col = 0
        hi_col = 0
        for b0, g in groups:
            lo_ids = []
            hi_ids = []
            for b in range(b0, b0 + g):
                ll = lo_lists[r][b]
                ll = ll + [0] * (c_lo * P - len(ll))
                lo_ids.extend(ll)
                hl = hi_lists[r][b]
                hl = hl + [0] * (c_hi * P - len(hl))
                hi_ids.extend(hl)
                # dst-local ids, chunk-major (lo chunks then hi chunks)
                dl_pad_lo = lo_dl[r][b] + [300] * (c_lo * P - len(lo_dl[r][b]))
                dl_pad_hi = hi_dl[r][b] + [300] * (c_hi * P - len(hi_dl[r][b]))
                dl_all = dl_pad_lo + dl_pad_hi
                for c in range(cpb):
                    dl_arr[r, :, b * cpb + c] = dl_all[c * P : (c + 1) * P]
            w = wrap_idx(lo_ids)
            idx_lo[r][:, lo_col : lo_col + w.shape[1]] = w
            lo_col += w.shape[1]
            if c_hi > 0:
                w = wrap_idx(hi_ids)
                idx_hi[r][:, hi_col : hi_col + w.shape[1]] = w
                hi_col += w.shape[1]

    dinv_own = np.zeros((n_cores, P, nblk), dtype=np.float32)
    for r in range(n_cores):
        own = dinv[r * npc : (r + 1) * npc]
        own = np.pad(own, (0, npc_pad - npc))
        dinv_own[r] = own.reshape(nblk, P).T

    return dict(
        n_cores=n_cores,
        N=N,
        E=E,
        half=half,
        npc=npc,
        nblk=nblk,
        npc_pad=npc_pad,
        c_lo=c_lo,
        c_hi=c_hi,
        groups=groups,
        idx_lo=idx_lo,
        idx_hi=idx_hi,
        dl=dl_arr,
        dinv_own=dinv_own,
    )


# ----------------------------------------------------------------------------
# Kernel builder (same BIR for all cores; per-core data via input tensors)
# ----------------------------------------------------------------------------
def build_kernel(plan, DIN, F1, F2, F3, skip=(), repeat=1, gbufs=3):
    n_cores = plan["n_cores"]
    N = plan["N"]
    half = plan["half"]
    nblk = plan["nblk"]
    npc = plan["npc"]
    npc_pad = plan["npc_pad"]
    c_lo = plan["c_lo"]
    c_hi = plan["c_hi"]
    cpb = c_lo + c_hi
    groups = plan["groups"]
    rows_total = n_cores * npc_pad
    rg = [list(range(n_cores))]

    nc = bacc.Bacc("TRN2", target_bir_lowering=False, debug=False,
                   num_devices=n_cores, num_swdge_queues=4)
    import itertools
    _gq = itertools.count()

    # ---- I/O ----
    aT0 = nc.dram_tensor("aT0", [P, npc_pad], BF16, kind="ExternalInput")
    W1 = nc.dram_tensor("W1", [P, F1], BF16, kind="ExternalInput")
    W2 = nc.dram_tensor("W2", [P, F2], BF16, kind="ExternalInput")
    W3 = nc.dram_tensor("W3", [P, F3], BF16, kind="ExternalInput")
    g1 = nc.dram_tensor("g1", [P, 1], F32, kind="ExternalInput")
    be1 = nc.dram_tensor("be1", [P, 1], F32, kind="ExternalInput")
    g2 = nc.dram_tensor("g2", [P, 1], F32, kind="ExternalInput")
    be2 = nc.dram_tensor("be2", [P, 1], F32, kind="ExternalInput")
    b3b = nc.dram_tensor("b3b", [P, F3], F32, kind="ExternalInput")
    iota_in = nc.dram_tensor("iota", [P, P], BF16, kind="ExternalInput")
    ident_in = nc.dram_tensor("ident", [P, P], BF16, kind="ExternalInput")
    idx_lo_in = nc.dram_tensor("idx_lo", list(plan["idx_lo"].shape[1:]), I16,
                               kind="ExternalInput")
    idx_hi_in = nc.dram_tensor("idx_hi", list(plan["idx_hi"].shape[1:]), I16,
                               kind="ExternalInput")
    dl_in = nc.dram_tensor("dl", [P, nblk * cpb], BF16, kind="ExternalInput")
    dinv_in = nc.dram_tensor("dinv_own", [P, nblk], F32, kind="ExternalInput")
    out_t = nc.dram_tensor("out", [npc, F3], F32, kind="ExternalOutput")

    with tile.TileContext(nc) as tc, ExitStack() as ctx:
        nc.gpsimd.load_library(library_config.mlp)

        sb = ctx.enter_context(tc.tile_pool(name="sb", bufs=1))
        # persistent sbuf state
        aT_a = sb.tile([P, npc_pad], BF16, tag="aT_a")
        aT_b = sb.tile([P, npc_pad], BF16, tag="aT_b")
        u_own = sb.tile([P, nblk, max(F1, F2)], BF16, tag="u_own")
        u_own3 = sb.tile([P, nblk, P], BF16, tag="u_own3")
        z_own = sb.tile([P, nblk, max(F1, F2)], BF16, tag="z_own")
        w_sb = sb.tile([P, F1 + F2 + F3], BF16, tag="w_sb")
        iota_t = sb.tile([P, P], BF16, tag="iota_t")
        ident_t = sb.tile([P, P], BF16, tag="ident_t")
        ones_t = sb.tile([P, 1], BF16, tag="ones_t")
        dinv_t = sb.tile([P, nblk], F32, tag="dinv_t")
        dl_t = sb.tile([P, nblk * cpb], BF16, tag="dl_t")
        ilo_t = sb.tile(list(plan["idx_lo"].shape[1:]), I16, tag="ilo_t")
        ihi_t = sb.tile(list(plan["idx_hi"].shape[1:]), I16, tag="ihi_t")
        bnp_t = sb.tile([P, 4], F32, tag="bnp_t")  # g1 be1 g2 be2
        b3_t = sb.tile([P, F3], F32, tag="b3_t")

        nc.sync.dma_start(w_sb[:, 0:F1], W1[:])
        nc.sync.dma_start(w_sb[:, F1:F1 + F2], W2[:])
        nc.sync.dma_start(w_sb[:, F1 + F2:], W3[:])
        nc.sync.dma_start(iota_t[:], iota_in[:])
        nc.sync.dma_start(ident_t[:], ident_in[:])
        nc.sync.dma_start(dinv_t[:], dinv_in[:])
        nc.sync.dma_start(dl_t[:], dl_in[:])
        nc.sync.dma_start(ilo_t[:], idx_lo_in[:])
        if c_hi > 0:
            nc.sync.dma_start(ihi_t[:], idx_hi_in[:])
        nc.sync.dma_start(bnp_t[:, 0:1], g1[:])
        nc.sync.dma_start(bnp_t[:, 1:2], be1[:])
        nc.sync.dma_start(bnp_t[:, 2:3], g2[:])
        nc.sync.dma_start(bnp_t[:, 3:4], be2[:])
        nc.sync.dma_start(b3_t[:], b3b[:])
        nc.sync.dma_start(aT_a[:], aT0[:])
        nc.gpsimd.memset(ones_t[:], 1.0)
        nc.gpsimd.memset(u_own3[:, :, F3:], 0.0)

        # DRAM scratch
        dram = ctx.enter_context(tc.tile_pool(name="dram", bufs=1,
                                              space="DRAM"))
        u1_dram = dram.tile([npc_pad, F1], BF16, tag="u1")
        u2_dram = dram.tile([npc_pad, F2], BF16, tag="u2")
        u3_dram = dram.tile([npc_pad, P], BF16, tag="u3")
        st_in1 = dram.tile([P, 2], F32, tag="st_in1")
        st_in2 = dram.tile([P, 2], F32, tag="st_in2")

        # working pools
        psum_mm = ctx.enter_context(
            tc.tile_pool(name="psum_mm", bufs=2, space="PSUM"))
        psum_agg = ctx.enter_context(
            tc.tile_pool(name="psum_agg", bufs=2, space="PSUM"))
        psum_st = ctx.enter_context(
            tc.tile_pool(name="psum_st", bufs=2, space="PSUM"))
        spool = ctx.enter_context(tc.tile_pool(name="spool", bufs=4))
        gpool = ctx.enter_context(tc.tile_pool(name="gpool", bufs=gbufs))
        tpool = ctx.enter_context(tc.tile_pool(name="tpool", bufs=3))

        gbmax = max(g for _, g in groups)

        def layer(l, aT_in, aT_out, F_in, F_out, w_off, u_dram, ufull,
                  is_last, g_col=None, be_col=None, st_in=None, st_out=None):
            # ---------------- Phase A: dense matmul + u table ----------
            for b in range(nblk):
                h_ps = psum_mm.tile([P, F_out], F32, tag="mm")
                nc.tensor.matmul(
                    h_ps[:],
                    lhsT=aT_in[:, b * P:(b + 1) * P],
                    rhs=w_sb[:, w_off:w_off + F_out],
                    start=True, stop=True,
                )
                uo = u_own3 if is_last else u_own
                nc.scalar.activation(uo[:, b, :F_out], h_ps[:], AF.Copy,
                                     scale=dinv_t[:, b:b + 1])
            uo = u_own3 if is_last else u_own
            wtab = P if is_last else F_out
            nc.sync.dma_start(
                u_dram[:].rearrange("(b p) f -> p b f", p=P),
                uo[:, :, :wtab],
            )
            if n_cores > 1 and "coll" not in skip:
                nc.gpsimd.collective_compute(
                    "AllGather", ALU.bypass, replica_groups=rg,
                    ins=[u_dram[:].opt()], outs=[ufull[:].opt()],
                )
            else:
                nc.sync.dma_start(ufull[0:npc_pad, :], u_dram[:])

            # ---------------- Phase B: gather + segment matmul ---------
            if not is_last:
                st_s = psum_st.tile([P, 1], F32, tag="st_s")
                st_q = psum_st.tile([P, 1], F32, tag="st_q")
            lo_col = 0
            hi_col = 0
            for b0, g in groups:
                n_lo = g * c_lo * P
                lo_t = gpool.tile([P, gbmax * c_lo, P], BF16, tag="lo")
                if "noload" in skip:
                    pass
                elif "seqload" in skip:
                    nc.gpsimd.dma_start(
                        lo_t[:, :g * c_lo, :],
                        ufull[0:n_lo, :].rearrange(
                            "(c p) f -> p c f", p=P),
                    )
                else:
                    nc.gpsimd.dma_gather(
                        lo_t[:, :g * c_lo, :], ufull[0:half, :],
                        ilo_t[:, lo_col:lo_col + n_lo // 16],
                        n_lo, n_lo, P, single_packet=False,
                        queue_num=next(_gq) % 4,
                    )
                lo_col += n_lo // 16
                if c_hi > 0:
                    n_hi = g * c_hi * P
                    hi_t = gpool.tile([P, gbmax * c_hi, P], BF16, tag="hi")
                    if "noload" in skip:
                        pass
                    elif "seqload" in skip:
                        nc.gpsimd.dma_start(
                            hi_t[:, :g * c_hi, :],
                            ufull[half:half + n_hi, :].rearrange(
                                "(c p) f -> p c f", p=P),
                        )
                    else:
                        nc.gpsimd.dma_gather(
                            hi_t[:, :g * c_hi, :],
                            ufull[half:rows_total, :],
                            ihi_t[:, hi_col:hi_col + n_hi // 16],
                            n_hi, n_hi, P, single_packet=False,
                            queue_num=next(_gq) % 4,
                        )
                    hi_col += n_hi // 16
                for bb in range(g):
                    b = b0 + bb
                    agg = psum_agg.tile([P, F_out], F32, tag="agg")
                    if "seg" in skip:
                        nc.vector.memset(agg[:], 0.0)
                    else:
                        s_w = spool.tile([P, cpb, P], BF16, tag="s")
                        nc.vector.tensor_tensor(
                            out=s_w[:],
                            in0=iota_t[:, None, :].to_broadcast([P, cpb, P]),
                            in1=dl_t[:, b * cpb:(b + 1) * cpb].to_broadcast(
                                [P, cpb, P]),
                            op=ALU.is_equal,
                        )
                        for c in range(cpb):
                            if c < c_lo:
                                rhs = lo_t[:, bb * c_lo + c, :F_out]
                            else:
                                rhs = hi_t[:, bb * c_hi + (c - c_lo), :F_out]
                            nc.tensor.matmul(
                                agg[:], lhsT=s_w[:, c, :], rhs=rhs,
                                start=(c == 0), stop=(c == cpb - 1),
                            )
                    # epilogue: z = dinv * (agg + u_own)
                    uo = u_own3 if is_last else u_own
                    t_t = tpool.tile([P, F_out], F32, tag="t")
                    nc.vector.tensor_tensor(
                        out=t_t[:], in0=agg[:], in1=uo[:, b, :F_out],
                        op=ALU.add,
                    )
                    if is_last:
                        z3 = tpool.tile([P, F_out], F32, tag="z3")
                        nc.scalar.activation(z3[:], t_t[:], AF.Copy,
                                             scale=dinv_t[:, b:b + 1])
                        o_t = tpool.tile([P, F_out], F32, tag="o")
                        nc.vector.tensor_tensor(out=o_t[:], in0=z3[:],
                                                in1=b3_t[:], op=ALU.add)
                        hi_row = min(npc, (b + 1) * P) - b * P
                        nc.sync.dma_start(out_t[b * P:b * P + hi_row, :],
                                          o_t[:hi_row, :])
                    else:
                        nc.scalar.activation(z_own[:, b, :F_out], t_t[:],
                                             AF.Copy,
                                             scale=dinv_t[:, b:b + 1])
                        z2 = tpool.tile([P, F_out], BF16, tag="z2")
                        nc.scalar.activation(z2[:], z_own[:, b, :F_out],
                                             AF.Square)
                        nc.tensor.matmul(st_s[:], lhsT=z_own[:, b, :F_out],
                                         rhs=ones_t[:],
                                         start=(b == 0), stop=(b == nblk - 1),
                                         skip_group_check=True)
                        nc.tensor.matmul(st_q[:], lhsT=z2[:], rhs=ones_t[:],
                                         start=(b == 0), stop=(b == nblk - 1),
                                         skip_group_check=True)
            if is_last:
                return

            # ---------------- Phase C: BN stats allreduce + coeffs -----
            st_sb = tpool.tile([P, 2], F32, tag="stsb")
            nc.vector.tensor_copy(st_sb[:, 0:1], st_s[:])
            nc.vector.tensor_copy(st_sb[:, 1:2], st_q[:])
            nc.sync.dma_start(st_in[:], st_sb[:])
            if n_cores > 1 and "coll" not in skip:
                nc.gpsimd.collective_compute(
                    "AllReduce", ALU.add, replica_groups=rg,
                    ins=[st_in[:].opt()], outs=[st_out[:].opt()],
                )
            else:
                nc.sync.dma_start(st_out[:], st_in[:])
            st_g = tpool.tile([P, 2], F32, tag="stg")
            nc.sync.dma_start(st_g[:], st_out[:])
            m_t = tpool.tile([P, 1], F32, tag="m")
            nc.scalar.activation(m_t[:], st_g[:, 0:1], AF.Copy, scale=1.0 / N)
            q_t = tpool.tile([P, 1], F32, tag="q")
            nc.scalar.activation(q_t[:], st_g[:, 1:2], AF.Copy, scale=1.0 / N)
            m2_t = tpool.tile([P, 1], F32, tag="m2")
            nc.scalar.activation(m2_t[:], m_t[:], AF.Square)
            v_t = tpool.tile([P, 1], F32, tag="v")
            nc.vector.tensor_tensor(out=v_t[:], in0=q_t[:], in1=m2_t[:],
                                    op=ALU.subtract)
            ve_t = tpool.tile([P, 1], F32, tag="ve")
            nc.vector.tensor_scalar(out=ve_t[:], in0=v_t[:], scalar1=1e-5,
                                    scalar2=None, op0=ALU.add)
            sd_t = tpool.tile([P, 1], F32, tag="sd")
            nc.scalar.activation(sd_t[:], ve_t[:], AF.Sqrt)
            inv_t = tpool.tile([P, 1], F32, tag="inv")
            nc.vector.reciprocal(inv_t[:], sd_t[:])
            a_t = tpool.tile([P, 1], F32, tag="A")
            nc.vector.tensor_tensor(out=a_t[:], in0=bnp_t[:, g_col:g_col + 1],
                                    in1=inv_t[:], op=ALU.mult)
            ma_t = tpool.tile([P, 1], F32, tag="mA")
            nc.vector.tensor_tensor(out=ma_t[:], in0=m_t[:], in1=a_t[:],
                                    op=ALU.mult)
            bb_t = tpool.tile([P, 1], F32, tag="B")
            nc.vector.tensor_tensor(out=bb_t[:],
                                    in0=bnp_t[:, be_col:be_col + 1],
                                    in1=ma_t[:], op=ALU.subtract)

            # ---------------- Phase D: transpose + BN apply + relu -----
            for b in range(nblk):
                zT = psum_mm.tile([P, P], BF16, tag="mm")
                nc.tensor.transpose(zT[:], z_own[:, b, :F_out], ident_t[:])
                nc.scalar.activation(aT_out[:, b * P:(b + 1) * P], zT[:],
                                     AF.Relu, bias=bb_t[:], scale=a_t[:])

        for _rep in range(repeat):
            ufull1 = dram.tile([rows_total, F1], BF16, tag=f"uf1_{_rep}",
                               addr_space="Shared")
            ufull2 = dram.tile([rows_total, F2], BF16, tag=f"uf2_{_rep}",
                               addr_space="Shared")
            ufull3 = dram.tile([rows_total, P], BF16, tag=f"uf3_{_rep}",
                               addr_space="Shared")
            st_out1 = dram.tile([P, 2], F32, tag=f"st_out1_{_rep}",
                                addr_space="Shared")
            st_out2 = dram.tile([P, 2], F32, tag=f"st_out2_{_rep}",
                                addr_space="Shared")
            if _rep > 0:
                nc.sync.dma_start(aT_a[:], aT0[:])
            layer(1, aT_a, aT_b, DIN, F1, 0, u1_dram, ufull1, False, 0, 1,
                  st_in1, st_out1)
            layer(2, aT_b, aT_a, F1, F2, F1, u2_dram, ufull2, False, 2, 3,
                  st_in2, st_out2)
            layer(3, aT_a, None, F2, F3, F1 + F2, u3_dram, ufull3, True)

    nc.compile()
    return nc


# ----------------------------------------------------------------------------
# Host entry point
# ----------------------------------------------------------------------------
def make_in_maps(plan, inputs, DIN, F1, F2, F3):
    n_cores = plan["n_cores"]
    npc = plan["npc"]
    npc_pad = plan["npc_pad"]
    x = np.asarray(inputs["x"], dtype=np.float32)
    import ml_dtypes
    bf16 = ml_dtypes.bfloat16
    iota = np.tile(np.arange(P)[None, :], (P, 1)).astype(bf16)
    ident = np.eye(P, dtype=bf16)
    b3b = np.tile(np.asarray(inputs["b3"], np.float32)[None, :], (P, 1))
    col = lambda v: np.asarray(v, np.float32).reshape(P, 1)
    in_maps = []
    for r in range(n_cores):
        xr = x[r * npc:(r + 1) * npc]
        aT0 = np.zeros((P, npc_pad), bf16)
        aT0[:, :npc] = xr.T.astype(bf16)
        in_maps.append({
            "aT0": aT0,
            "W1": np.asarray(inputs["W1"], np.float32).astype(bf16),
            "W2": np.asarray(inputs["W2"], np.float32).astype(bf16),
            "W3": np.asarray(inputs["W3"], np.float32).astype(bf16),
            "g1": col(inputs["g1"]), "be1": col(inputs["be1"]),
            "g2": col(inputs["g2"]), "be2": col(inputs["be2"]),
            "b3b": b3b, "iota": iota, "ident": ident,
            "idx_lo": plan["idx_lo"][r], "idx_hi": plan["idx_hi"][r],
            "dl": plan["dl"][r].astype(ml_dtypes.bfloat16), "dinv_own": plan["dinv_own"][r],
        })
    return in_maps


_CACHE = {}


def kernel(**inputs):
    x = np.asarray(inputs["x"], dtype=np.float32)
    N, DIN = x.shape
    F1 = inputs["W1"].shape[1]
    F2 = inputs["W2"].shape[1]
    F3 = inputs["W3"].shape[1]
    edge_index = np.asarray(inputs["edge_index"])

    key = (N, DIN, F1, F2, F3, hash(edge_index.tobytes()))
    if key not in _CACHE:
        plan = make_plan(edge_index, N)
        nc = build_kernel(plan, DIN, F1, F2, F3)
        _CACHE[key] = (plan, nc)
    plan, nc = _CACHE[key]

    in_maps = make_in_maps(plan, inputs, DIN, F1, F2, F3)
    res = run_bass_kernel_spmd(nc, in_maps, core_ids=list(range(plan["n_cores"])))
    out = np.concatenate([res.results[r]["out"] for r in range(plan["n_cores"])],
                         axis=0)
    return out.astype(np.float32)


if __name__ == "__main__":
    import reference

    inputs = {k: np.asarray(v) for k, v in reference.setup_inputs().items()}
    out = kernel(**inputs)
    exp = np.asarray(reference.reference(**inputs))
    err = np.abs(out - exp).max() / (np.abs(exp).max() + 1e-30)
    print("Relative error:", err)



# revision 35
# speedup vs baseline: 1.0305x; 1.0305x over previous
"""Trainium2 Bass kernel for a 3-layer GCN (DeepGRL) on 8 NeuronCores.

Strategy (per the dst-partitioned sharding plan):
  - Nodes are sharded contiguously across the 8 cores; edges are owned by the
    core that owns their destination node.
  - Per layer:  h = a @ W  (dense matmul on PE, per-core own nodes),
    u = dinv * h is written to a DRAM table and AllGather'ed so every core
    holds the full [N, F] table.
  - Aggregation out_i = dinv_i * (sum_{e: dst=i} u[src_e] + u_i) + b is done
    per 128-dst-node block: edge source rows are fetched with the SWDGE
    dma_gather instruction (128 rows per chunk land on 128 partitions), and a
    one-hot "segment matrix" S (built on the vector engine from the dst-local
    ids with an is_equal compare against an iota row) maps edges to dst rows
    via a PE matmul accumulating in PSUM.
  - BatchNorm batch statistics (sum / sum-of-squares per feature) are computed
    with ones-vector matmuls and AllReduce'd across cores; BN apply + ReLU is
    fused into one scalar-engine activation during the transpose back to the
    feature-major layout the next layer's matmul needs.

dma_gather indices are int16, so the gathered table is addressed in two
halves (rows < HALF and rows >= HALF); every dst block's edge list is split
into a "lo" and a "hi" sublist, each padded to a multiple of 128.
"""

import math
from contextlib import ExitStack

import numpy as np

import concourse.bacc as bacc
import concourse.bass as bass
import concourse.mybir as mybir
import concourse.tile as tile
from concourse import library_config
from concourse.bass_utils import run_bass_kernel_spmd

P = 128
F32 = mybir.dt.float32
BF16 = mybir.dt.bfloat16
I16 = mybir.dt.int16
AF = mybir.ActivationFunctionType
ALU = mybir.AluOpType


# ----------------------------------------------------------------------------
# Host-side graph preprocessing
# ----------------------------------------------------------------------------
def make_plan(edge_index, N, n_cores=8, nblkA=25, gb=4):
    """Partition edges by destination core, build per-core gather index /
    segment-id arrays (compile-time constants of the kernel).

    The u table is split into two AllGather'd halves by per-core block
    range: blocks [0, nblkA) -> table A, blocks [nblkA, nblk) -> table B.
    Gathers from table A can start as soon as AllGather-A lands, while
    AllGather-B is still in flight.  (lo == A, hi == B throughout.)"""
    src = np.asarray(edge_index[0], dtype=np.int64)
    dst = np.asarray(edge_index[1], dtype=np.int64)
    E = src.shape[0]

    indeg = np.bincount(dst, minlength=N).astype(np.float64)
    deg = indeg + 1.0  # self loop
    dinv = (1.0 / np.sqrt(deg)).astype(np.float32)

    npc = N // n_cores
    assert npc * n_cores == N
    nblk = math.ceil(npc / P)
    npc_pad = nblk * P
    nblkB = nblk - nblkA
    rA = nblkA * P  # per-core rows in table A
    rB = nblkB * P
    assert n_cores * rA < 32768 and n_cores * rB < 32768  # int16 gather idx

    dst_core = dst // npc
    dloc = dst - dst_core * npc

    # source row in the two-table layout
    src_core = src // npc
    src_loc = src - src_core * npc

    # bucket edges: per (core, block) -> lo (table A) / hi (table B) list
    lo_lists = [[[] for _ in range(nblk)] for _ in range(n_cores)]
    hi_lists = [[[] for _ in range(nblk)] for _ in range(n_cores)]
    lo_dl = [[[] for _ in range(nblk)] for _ in range(n_cores)]
    hi_dl = [[[] for _ in range(nblk)] for _ in range(n_cores)]
    order = np.argsort(dst, kind="stable")
    for e in order:
        r = dst_core[e]
        b = dloc[e] // P
        d_in_blk = dloc[e] - b * P
        sl = src_loc[e]
        if sl < rA:
            lo_lists[r][b].append(src_core[e] * rA + sl)
            lo_dl[r][b].append(d_in_blk)
        else:
            hi_lists[r][b].append(src_core[e] * rB + (sl - rA))
            hi_dl[r][b].append(d_in_blk)

    c_lo = max(
        1,
        max(
            math.ceil(len(lo_lists[r][b]) / P)
            for r in range(n_cores)
            for b in range(nblk)
        ),
    )
    n_hi = max(
        len(hi_lists[r][b]) for r in range(n_cores) for b in range(nblk)
    )
    c_hi = math.ceil(n_hi / P)  # may be 0
    cpb = c_lo + c_hi  # chunks per block

    # gather groups of up to `gb` blocks
    groups = []
    b0 = 0
    while b0 < nblk:
        g = min(gb, nblk - b0)
        groups.append((b0, g))
        b0 += g

    def wrap_idx(ids):
        """int16 wrap layout: idx i -> [i % 16, i // 16], replicated to 128
        partitions (8 groups of 16)."""
        ids = np.asarray(ids, dtype=np.int16)
        L = ids.shape[0]
        assert L % 16 == 0
        w = ids.reshape(L // 16, 16).T  # [16, L/16]
        return np.tile(w, (8, 1))  # [128, L/16]

    idx_lo = np.zeros((n_cores, P, nblk * c_lo * 8), dtype=np.int16)
    idx_hi = np.zeros((n_cores, P, max(1, nblk * c_hi * 8)), dtype=np.int16)
    dl_arr = np.full((n_cores, P, nblk * cpb), 300.0, dtype=np.float32)

    for r in range(n_cores):
        lo_col = 0
        hi_col = 0
        for b0, g in groups:
            lo_ids = []
            hi_ids = []
            for b in range(b0, b0 + g):
                ll = lo_lists[r][b]
                ll = ll + [0] * (c_lo * P - len(ll))
                lo_ids.extend(ll)
                hl = hi_lists[r][b]
                hl = hl + [0] * (c_hi * P - len(hl))
                hi_ids.extend(hl)
                # dst-local ids, chunk-major (lo chunks then hi chunks)
                dl_pad_lo = lo_dl[r][b] + [300] * (c_lo * P - len(lo_dl[r][b]))
                dl_pad_hi = hi_dl[r][b] + [300] * (c_hi * P - len(hi_dl[r][b]))
                dl_all = dl_pad_lo + dl_pad_hi
                for c in range(cpb):
                    dl_arr[r, :, b * cpb + c] = dl_all[c * P : (c + 1) * P]
            w = wrap_idx(lo_ids)
            idx_lo[r][:, lo_col : lo_col + w.shape[1]] = w
            lo_col += w.shape[1]
            if c_hi > 0:
                w = wrap_idx(hi_ids)
                idx_hi[r][:, hi_col : hi_col + w.shape[1]] = w
                hi_col += w.shape[1]

    dinv_own = np.zeros((n_cores, P, nblk), dtype=np.float32)
    for r in range(n_cores):
        own = dinv[r * npc : (r + 1) * npc]
        own = np.pad(own, (0, npc_pad - npc))
        dinv_own[r] = own.reshape(nblk, P).T

    return dict(
        n_cores=n_cores,
        N=N,
        E=E,
        nblkA=nblkA,
        rA=rA,
        rB=rB,
        npc=npc,
        nblk=nblk,
        npc_pad=npc_pad,
        c_lo=c_lo,
        c_hi=c_hi,
        groups=groups,
        idx_lo=idx_lo,
        idx_hi=idx_hi,
        dl=dl_arr,
        dinv_own=dinv_own,
    )


# ----------------------------------------------------------------------------
# Kernel builder (same BIR for all cores; per-core data via input tensors)
# ----------------------------------------------------------------------------
def build_kernel(plan, DIN, F1, F2, F3, skip=(), repeat=1, gbufs=3):
    n_cores = plan["n_cores"]
    N = plan["N"]
    nblkA = plan["nblkA"]
    rA = plan["rA"]
    rB = plan["rB"]
    nblk = plan["nblk"]
    npc = plan["npc"]
    npc_pad = plan["npc_pad"]
    c_lo = plan["c_lo"]
    c_hi = plan["c_hi"]
    cpb = c_lo + c_hi
    groups = plan["groups"]
    rtA = n_cores * rA
    rtB = n_cores * rB
    rg = [list(range(n_cores))]

    nc = bacc.Bacc("TRN2", target_bir_lowering=False, debug=False,
                   num_devices=n_cores, num_swdge_queues=4)
    import itertools
    _gq = itertools.count()

    # ---- I/O ----
    aT0 = nc.dram_tensor("aT0", [P, npc_pad], BF16, kind="ExternalInput")
    W1 = nc.dram_tensor("W1", [P, F1], BF16, kind="ExternalInput")
    W2 = nc.dram_tensor("W2", [P, F2], BF16, kind="ExternalInput")
    W3 = nc.dram_tensor("W3", [P, F3], BF16, kind="ExternalInput")
    g1 = nc.dram_tensor("g1", [P, 1], F32, kind="ExternalInput")
    be1 = nc.dram_tensor("be1", [P, 1], F32, kind="ExternalInput")
    g2 = nc.dram_tensor("g2", [P, 1], F32, kind="ExternalInput")
    be2 = nc.dram_tensor("be2", [P, 1], F32, kind="ExternalInput")
    b3b = nc.dram_tensor("b3b", [P, F3], F32, kind="ExternalInput")
    iota_in = nc.dram_tensor("iota", [P, P], BF16, kind="ExternalInput")
    ident_in = nc.dram_tensor("ident", [P, P], BF16, kind="ExternalInput")
    idx_lo_in = nc.dram_tensor("idx_lo", list(plan["idx_lo"].shape[1:]), I16,
                               kind="ExternalInput")
    idx_hi_in = nc.dram_tensor("idx_hi", list(plan["idx_hi"].shape[1:]), I16,
                               kind="ExternalInput")
    dl_in = nc.dram_tensor("dl", [P, nblk * cpb], BF16, kind="ExternalInput")
    dinv_in = nc.dram_tensor("dinv_own", [P, nblk], F32, kind="ExternalInput")
    out_t = nc.dram_tensor("out", [npc, F3], F32, kind="ExternalOutput")

    with tile.TileContext(nc) as tc, ExitStack() as ctx:
        nc.gpsimd.load_library(library_config.mlp)

        sb = ctx.enter_context(tc.tile_pool(name="sb", bufs=1))
        # persistent sbuf state
        aT_a = sb.tile([P, npc_pad], BF16, tag="aT_a")
        aT_b = sb.tile([P, npc_pad], BF16, tag="aT_b")
        u_own = sb.tile([P, nblk, max(F1, F2)], BF16, tag="u_own")
        u_own3 = sb.tile([P, nblk, P], BF16, tag="u_own3")
        z_own = sb.tile([P, nblk, max(F1, F2)], BF16, tag="z_own")
        w_sb = sb.tile([P, F1 + F2 + F3], BF16, tag="w_sb")
        iota_t = sb.tile([P, P], BF16, tag="iota_t")
        ident_t = sb.tile([P, P], BF16, tag="ident_t")
        ones_t = sb.tile([P, 1], BF16, tag="ones_t")
        dinv_t = sb.tile([P, nblk], F32, tag="dinv_t")
        dl_t = sb.tile([P, nblk * cpb], BF16, tag="dl_t")
        ilo_t = sb.tile(list(plan["idx_lo"].shape[1:]), I16, tag="ilo_t")
        ihi_t = sb.tile(list(plan["idx_hi"].shape[1:]), I16, tag="ihi_t")
        bnp_t = sb.tile([P, 4], F32, tag="bnp_t")  # g1 be1 g2 be2
        b3_t = sb.tile([P, F3], F32, tag="b3_t")

        nc.sync.dma_start(w_sb[:, 0:F1], W1[:])
        nc.sync.dma_start(w_sb[:, F1:F1 + F2], W2[:])
        nc.sync.dma_start(w_sb[:, F1 + F2:], W3[:])
        nc.sync.dma_start(iota_t[:], iota_in[:])
        nc.sync.dma_start(ident_t[:], ident_in[:])
        nc.sync.dma_start(dinv_t[:], dinv_in[:])
        nc.sync.dma_start(dl_t[:], dl_in[:])
        nc.sync.dma_start(ilo_t[:], idx_lo_in[:])
        if c_hi > 0:
            nc.sync.dma_start(ihi_t[:], idx_hi_in[:])
        nc.sync.dma_start(bnp_t[:, 0:1], g1[:])
        nc.sync.dma_start(bnp_t[:, 1:2], be1[:])
        nc.sync.dma_start(bnp_t[:, 2:3], g2[:])
        nc.sync.dma_start(bnp_t[:, 3:4], be2[:])
        nc.sync.dma_start(b3_t[:], b3b[:])
        nc.sync.dma_start(aT_a[:], aT0[:])
        nc.gpsimd.memset(ones_t[:], 1.0)
        nc.gpsimd.memset(u_own3[:, :, F3:], 0.0)

        # DRAM scratch
        dram = ctx.enter_context(tc.tile_pool(name="dram", bufs=1,
                                              space="DRAM"))
        u1A = dram.tile([rA, F1], BF16, tag="u1A")
        u1B = dram.tile([rB, F1], BF16, tag="u1B")
        u2A = dram.tile([rA, F2], BF16, tag="u2A")
        u2B = dram.tile([rB, F2], BF16, tag="u2B")
        u3A = dram.tile([rA, P], BF16, tag="u3A")
        u3B = dram.tile([rB, P], BF16, tag="u3B")
        st_in1 = dram.tile([P, 2], F32, tag="st_in1")
        st_in2 = dram.tile([P, 2], F32, tag="st_in2")

        # working pools
        psum_mm = ctx.enter_context(
            tc.tile_pool(name="psum_mm", bufs=2, space="PSUM"))
        psum_agg = ctx.enter_context(
            tc.tile_pool(name="psum_agg", bufs=2, space="PSUM"))
        psum_st = ctx.enter_context(
            tc.tile_pool(name="psum_st", bufs=2, space="PSUM"))
        spool = ctx.enter_context(tc.tile_pool(name="spool", bufs=4))
        gpool = ctx.enter_context(tc.tile_pool(name="gpool", bufs=gbufs))
        tpool = ctx.enter_context(tc.tile_pool(name="tpool", bufs=3))

        gbmax = max(g for _, g in groups)

        def layer(l, aT_in, aT_out, F_in, F_out, w_off, udA, udB, ufA, ufB,
                  is_last, g_col=None, be_col=None, st_in=None, st_out=None):
            # ---------------- Phase A: dense matmul + u table ----------
            uo = u_own3 if is_last else u_own
            wtab = P if is_last else F_out

            def emit_half(ud, uf, b0, nb):
                nc.sync.dma_start(
                    ud[:].rearrange("(b p) f -> p b f", p=P),
                    uo[:, b0:b0 + nb, :wtab],
                )
                if n_cores > 1 and "coll" not in skip:
                    nc.gpsimd.collective_compute(
                        "AllGather", ALU.bypass, replica_groups=rg,
                        ins=[ud[:].opt()], outs=[uf[:].opt()],
                    )
                else:
                    nc.sync.dma_start(uf[0:nb * P, :], ud[:])

            for b in range(nblk):
                h_ps = psum_mm.tile([P, F_out], F32, tag="mm")
                nc.tensor.matmul(
                    h_ps[:],
                    lhsT=aT_in[:, b * P:(b + 1) * P],
                    rhs=w_sb[:, w_off:w_off + F_out],
                    start=True, stop=True,
                )
                nc.scalar.activation(uo[:, b, :F_out], h_ps[:], AF.Copy,
                                     scale=dinv_t[:, b:b + 1])
                if b == nblkA - 1:
                    emit_half(udA, ufA, 0, nblkA)
            emit_half(udB, ufB, nblkA, nblk - nblkA)

            # ---------------- Phase B: gather + segment matmul ---------
            if not is_last:
                st_s = psum_st.tile([P, 1], F32, tag="st_s")
                st_q = psum_st.tile([P, 1], F32, tag="st_q")
            lo_col = 0
            hi_col = 0
            for b0, g in groups:
                n_lo = g * c_lo * P
                lo_t = gpool.tile([P, gbmax * c_lo, P], BF16, tag="lo")
                if "noload" in skip:
                    pass
                elif "seqload" in skip:
                    nc.gpsimd.dma_start(
                        lo_t[:, :g * c_lo, :],
                        ufA[0:n_lo, :].rearrange(
                            "(c p) f -> p c f", p=P),
                    )
                else:
                    nc.gpsimd.dma_gather(
                        lo_t[:, :g * c_lo, :], ufA[:],
                        ilo_t[:, lo_col:lo_col + n_lo // 16],
                        n_lo, n_lo, P, single_packet=False,
                        queue_num=next(_gq) % 4,
                    )
                lo_col += n_lo // 16
                if c_hi > 0:
                    n_hi = g * c_hi * P
                    hi_t = gpool.tile([P, gbmax * c_hi, P], BF16, tag="hi")
                    if "noload" in skip:
                        pass
                    elif "seqload" in skip:
                        nc.gpsimd.dma_start(
                            hi_t[:, :g * c_hi, :],
                            ufB[0:n_hi, :].rearrange(
                                "(c p) f -> p c f", p=P),
                        )
                    else:
                        nc.gpsimd.dma_gather(
                            hi_t[:, :g * c_hi, :],
                            ufB[:],
                            ihi_t[:, hi_col:hi_col + n_hi // 16],
                            n_hi, n_hi, P, single_packet=False,
                            queue_num=next(_gq) % 4,
                        )
                    hi_col += n_hi // 16
                for bb in range(g):
                    b = b0 + bb
                    agg = psum_agg.tile([P, F_out], F32, tag="agg")
                    if "seg" in skip:
                        nc.vector.memset(agg[:], 0.0)
                    else:
                        s_w = spool.tile([P, cpb, P], BF16, tag="s")
                        nc.vector.tensor_tensor(
                            out=s_w[:],
                            in0=iota_t[:, None, :].to_broadcast([P, cpb, P]),
                            in1=dl_t[:, b * cpb:(b + 1) * cpb].to_broadcast(
                                [P, cpb, P]),
                            op=ALU.is_equal,
                        )
                        for c in range(cpb):
                            if c < c_lo:
                                rhs = lo_t[:, bb * c_lo + c, :F_out]
                            else:
                                rhs = hi_t[:, bb * c_hi + (c - c_lo), :F_out]
                            nc.tensor.matmul(
                                agg[:], lhsT=s_w[:, c, :], rhs=rhs,
                                start=(c == 0), stop=(c == cpb - 1),
                            )
                    # epilogue: z = dinv * (agg + u_own)
                    uo = u_own3 if is_last else u_own
                    t_t = tpool.tile([P, F_out], F32, tag="t")
                    nc.vector.tensor_tensor(
                        out=t_t[:], in0=agg[:], in1=uo[:, b, :F_out],
                        op=ALU.add,
                    )
                    if is_last:
                        z3 = tpool.tile([P, F_out], F32, tag="z3")
                        nc.scalar.activation(z3[:], t_t[:], AF.Copy,
                                             scale=dinv_t[:, b:b + 1])
                        o_t = tpool.tile([P, F_out], F32, tag="o")
                        nc.vector.tensor_tensor(out=o_t[:], in0=z3[:],
                                                in1=b3_t[:], op=ALU.add)
                        hi_row = min(npc, (b + 1) * P) - b * P
                        nc.sync.dma_start(out_t[b * P:b * P + hi_row, :],
                                          o_t[:hi_row, :])
                    else:
                        nc.scalar.activation(z_own[:, b, :F_out], t_t[:],
                                             AF.Copy,
                                             scale=dinv_t[:, b:b + 1])
                        z2 = tpool.tile([P, F_out], BF16, tag="z2")
                        nc.scalar.activation(z2[:], z_own[:, b, :F_out],
                                             AF.Square)
                        nc.tensor.matmul(st_s[:], lhsT=z_own[:, b, :F_out],
                                         rhs=ones_t[:],
                                         start=(b == 0), stop=(b == nblk - 1),
                                         skip_group_check=True)
                        nc.tensor.matmul(st_q[:], lhsT=z2[:], rhs=ones_t[:],
                                         start=(b == 0), stop=(b == nblk - 1),
                                         skip_group_check=True)
            if is_last:
                return

            # ---------------- Phase C: BN stats allreduce + coeffs -----
            st_sb = tpool.tile([P, 2], F32, tag="stsb")
            nc.vector.tensor_copy(st_sb[:, 0:1], st_s[:])
            nc.vector.tensor_copy(st_sb[:, 1:2], st_q[:])
            nc.sync.dma_start(st_in[:], st_sb[:])
            if n_cores > 1 and "coll" not in skip:
                nc.gpsimd.collective_compute(
                    "AllReduce", ALU.add, replica_groups=rg,
                    ins=[st_in[:].opt()], outs=[st_out[:].opt()],
                )
            else:
                nc.sync.dma_start(st_out[:], st_in[:])
            st_g = tpool.tile([P, 2], F32, tag="stg")
            nc.sync.dma_start(st_g[:], st_out[:])
            m_t = tpool.tile([P, 1], F32, tag="m")
            nc.scalar.activation(m_t[:], st_g[:, 0:1], AF.Copy, scale=1.0 / N)
            q_t = tpool.tile([P, 1], F32, tag="q")
            nc.scalar.activation(q_t[:], st_g[:, 1:2], AF.Copy, scale=1.0 / N)
            m2_t = tpool.tile([P, 1], F32, tag="m2")
            nc.scalar.activation(m2_t[:], m_t[:], AF.Square)
            v_t = tpool.tile([P, 1], F32, tag="v")
            nc.vector.tensor_tensor(out=v_t[:], in0=q_t[:], in1=m2_t[:],
                                    op=ALU.subtract)
            ve_t = tpool.tile([P, 1], F32, tag="ve")
            nc.vector.tensor_scalar(out=ve_t[:], in0=v_t[:], scalar1=1e-5,
                                    scalar2=None, op0=ALU.add)
            sd_t = tpool.tile([P, 1], F32, tag="sd")
            nc.scalar.activation(sd_t[:], ve_t[:], AF.Sqrt)
            inv_t = tpool.tile([P, 1], F32, tag="inv")
            nc.vector.reciprocal(inv_t[:], sd_t[:])
            a_t = tpool.tile([P, 1], F32, tag="A")
            nc.vector.tensor_tensor(out=a_t[:], in0=bnp_t[:, g_col:g_col + 1],
                                    in1=inv_t[:], op=ALU.mult)
            ma_t = tpool.tile([P, 1], F32, tag="mA")
            nc.vector.tensor_tensor(out=ma_t[:], in0=m_t[:], in1=a_t[:],
                                    op=ALU.mult)
            bb_t = tpool.tile([P, 1], F32, tag="B")
            nc.vector.tensor_tensor(out=bb_t[:],
                                    in0=bnp_t[:, be_col:be_col + 1],
                                    in1=ma_t[:], op=ALU.subtract)

            # ---------------- Phase D: transpose + BN apply + relu -----
            for b in range(nblk):
                zT = psum_mm.tile([P, P], BF16, tag="mm")
                nc.tensor.transpose(zT[:], z_own[:, b, :F_out], ident_t[:])
                nc.scalar.activation(aT_out[:, b * P:(b + 1) * P], zT[:],
                                     AF.Relu, bias=bb_t[:], scale=a_t[:])

        for _rep in range(repeat):
            uf1A = dram.tile([rtA, F1], BF16, tag=f"uf1A_{_rep}",
                             addr_space="Shared")
            uf1B = dram.tile([rtB, F1], BF16, tag=f"uf1B_{_rep}",
                             addr_space="Shared")
            uf2A = dram.tile([rtA, F2], BF16, tag=f"uf2A_{_rep}",
                             addr_space="Shared")
            uf2B = dram.tile([rtB, F2], BF16, tag=f"uf2B_{_rep}",
                             addr_space="Shared")
            uf3A = dram.tile([rtA, P], BF16, tag=f"uf3A_{_rep}",
                             addr_space="Shared")
            uf3B = dram.tile([rtB, P], BF16, tag=f"uf3B_{_rep}",
                             addr_space="Shared")
            st_out1 = dram.tile([P, 2], F32, tag=f"st_out1_{_rep}",
                                addr_space="Shared")
            st_out2 = dram.tile([P, 2], F32, tag=f"st_out2_{_rep}",
                                addr_space="Shared")
            if _rep > 0:
                nc.sync.dma_start(aT_a[:], aT0[:])
            layer(1, aT_a, aT_b, DIN, F1, 0, u1A, u1B, uf1A, uf1B, False,
                  0, 1, st_in1, st_out1)
            layer(2, aT_b, aT_a, F1, F2, F1, u2A, u2B, uf2A, uf2B, False,
                  2, 3, st_in2, st_out2)
            layer(3, aT_a, None, F2, F3, F1 + F2, u3A, u3B, uf3A, uf3B,
                  True)

    nc.compile()
    return nc


# ----------------------------------------------------------------------------
# Host entry point
# ----------------------------------------------------------------------------
def make_in_maps(plan, inputs, DIN, F1, F2, F3):
    n_cores = plan["n_cores"]
    npc = plan["npc"]
    npc_pad = plan["npc_pad"]
    x = np.asarray(inputs["x"], dtype=np.float32)
    import ml_dtypes
    bf16 = ml_dtypes.bfloat16
    iota = np.tile(np.arange(P)[None, :], (P, 1)).astype(bf16)
    ident = np.eye(P, dtype=bf16)
    b3b = np.tile(np.asarray(inputs["b3"], np.float32)[None, :], (P, 1))
    col = lambda v: np.asarray(v, np.float32).reshape(P, 1)
    in_maps = []
    for r in range(n_cores):
        xr = x[r * npc:(r + 1) * npc]
        aT0 = np.zeros((P, npc_pad), bf16)
        aT0[:, :npc] = xr.T.astype(bf16)
        in_maps.append({
            "aT0": aT0,
            "W1": np.asarray(inputs["W1"], np.float32).astype(bf16),
            "W2": np.asarray(inputs["W2"], np.float32).astype(bf16),
            "W3": np.asarray(inputs["W3"], np.float32).astype(bf16),
            "g1": col(inputs["g1"]), "be1": col(inputs["be1"]),
            "g2": col(inputs["g2"]), "be2": col(inputs["be2"]),
            "b3b": b3b, "iota": iota, "ident": ident,
            "idx_lo": plan["idx_lo"][r], "idx_hi": plan["idx_hi"][r],
            "dl": plan["dl"][r].astype(ml_dtypes.bfloat16), "dinv_own": plan["dinv_own"][r],
        })
    return in_maps


_CACHE = {}


def kernel(**inputs):
    x = np.asarray(inputs["x"], dtype=np.float32)
    N, DIN = x.shape
    F1 = inputs["W1"].shape[1]
    F2 = inputs["W2"].shape[1]
    F3 = inputs["W3"].shape[1]
    edge_index = np.asarray(inputs["edge_index"])

    key = (N, DIN, F1, F2, F3, hash(edge_index.tobytes()))
    if key not in _CACHE:
        plan = make_plan(edge_index, N)
        nc = build_kernel(plan, DIN, F1, F2, F3)
        _CACHE[key] = (plan, nc)
    plan, nc = _CACHE[key]

    in_maps = make_in_maps(plan, inputs, DIN, F1, F2, F3)
    res = run_bass_kernel_spmd(nc, in_maps, core_ids=list(range(plan["n_cores"])))
    out = np.concatenate([res.results[r]["out"] for r in range(plan["n_cores"])],
                         axis=0)
    return out.astype(np.float32)


if __name__ == "__main__":
    import reference

    inputs = {k: np.asarray(v) for k, v in reference.setup_inputs().items()}
    out = kernel(**inputs)
    exp = np.asarray(reference.reference(**inputs))
    err = np.abs(out - exp).max() / (np.abs(exp).max() + 1e-30)
    print("Relative error:", err)



# revision 37
# speedup vs baseline: 3.3169x; 3.2187x over previous
"""Trainium2 Bass kernel for a 3-layer GCN (DeepGRL) on 8 NeuronCores.

Strategy (per the dst-partitioned sharding plan):
  - Nodes are sharded contiguously across the 8 cores; edges are owned by the
    core that owns their destination node.
  - Per layer:  h = a @ W  (dense matmul on PE, per-core own nodes),
    u = dinv * h is written to a DRAM table and AllGather'ed so every core
    holds the full [N, F] table.
  - Aggregation out_i = dinv_i * (sum_{e: dst=i} u[src_e] + u_i) + b is done
    per 128-dst-node block: edge source rows are fetched with the SWDGE
    dma_gather instruction (128 rows per chunk land on 128 partitions), and a
    one-hot "segment matrix" S (built on the vector engine from the dst-local
    ids with an is_equal compare against an iota row) maps edges to dst rows
    via a PE matmul accumulating in PSUM.
  - BatchNorm batch statistics (sum / sum-of-squares per feature) are computed
    with ones-vector matmuls and AllReduce'd across cores; BN apply + ReLU is
    fused into one scalar-engine activation during the transpose back to the
    feature-major layout the next layer's matmul needs.

dma_gather indices are int16, so the gathered table is addressed in two
halves (rows < HALF and rows >= HALF); every dst block's edge list is split
into a "lo" and a "hi" sublist, each padded to a multiple of 128.
"""

import math
from contextlib import ExitStack

import numpy as np

import concourse.bacc as bacc
import concourse.bass as bass
import concourse.mybir as mybir
import concourse.tile as tile
from concourse import library_config
from concourse.bass_utils import run_bass_kernel_spmd

P = 128
F32 = mybir.dt.float32
BF16 = mybir.dt.bfloat16
I16 = mybir.dt.int16
AF = mybir.ActivationFunctionType
ALU = mybir.AluOpType


# ----------------------------------------------------------------------------
# Host-side graph preprocessing
# ----------------------------------------------------------------------------
def make_plan(edge_index, N, n_cores=8, nblkA=25, gb=4):
    """Partition edges by destination core, build per-core gather index /
    segment-id arrays (compile-time constants of the kernel).

    The u table is split into two AllGather'd halves by per-core block
    range: blocks [0, nblkA) -> table A, blocks [nblkA, nblk) -> table B.
    Gathers from table A can start as soon as AllGather-A lands, while
    AllGather-B is still in flight.  (lo == A, hi == B throughout.)"""
    src = np.asarray(edge_index[0], dtype=np.int64)
    dst = np.asarray(edge_index[1], dtype=np.int64)
    E = src.shape[0]

    indeg = np.bincount(dst, minlength=N).astype(np.float64)
    deg = indeg + 1.0  # self loop
    dinv = (1.0 / np.sqrt(deg)).astype(np.float32)

    npc = N // n_cores
    assert npc * n_cores == N
    nblk = math.ceil(npc / P)
    npc_pad = nblk * P
    nblkB = nblk - nblkA
    rA = nblkA * P  # per-core rows in table A
    rB = nblkB * P
    assert n_cores * rA < 32768 and n_cores * rB < 32768  # int16 gather idx

    dst_core = dst // npc
    dloc = dst - dst_core * npc

    # source row in the two-table layout
    src_core = src // npc
    src_loc = src - src_core * npc

    # bucket edges: per (core, block) -> lo (table A) / hi (table B) list
    lo_lists = [[[] for _ in range(nblk)] for _ in range(n_cores)]
    hi_lists = [[[] for _ in range(nblk)] for _ in range(n_cores)]
    lo_dl = [[[] for _ in range(nblk)] for _ in range(n_cores)]
    hi_dl = [[[] for _ in range(nblk)] for _ in range(n_cores)]
    order = np.argsort(dst, kind="stable")
    for e in order:
        r = dst_core[e]
        b = dloc[e] // P
        d_in_blk = dloc[e] - b * P
        sl = src_loc[e]
        if sl < rA:
            lo_lists[r][b].append(src_core[e] * rA + sl)
            lo_dl[r][b].append(d_in_blk)
        else:
            hi_lists[r][b].append(src_core[e] * rB + (sl - rA))
            hi_dl[r][b].append(d_in_blk)

    c_lo = max(
        1,
        max(
            math.ceil(len(lo_lists[r][b]) / P)
            for r in range(n_cores)
            for b in range(nblk)
        ),
    )
    n_hi = max(
        len(hi_lists[r][b]) for r in range(n_cores) for b in range(nblk)
    )
    c_hi = math.ceil(n_hi / P)  # may be 0
    cpb = c_lo + c_hi  # chunks per block

    # gather groups of up to `gb` blocks
    groups = []
    b0 = 0
    while b0 < nblk:
        g = min(gb, nblk - b0)
        groups.append((b0, g))
        b0 += g

    def wrap_idx(ids):
        """int16 wrap layout: idx i -> [i % 16, i // 16], replicated to 128
        partitions (8 groups of 16)."""
        ids = np.asarray(ids, dtype=np.int16)
        L = ids.shape[0]
        assert L % 16 == 0
        w = ids.reshape(L // 16, 16).T  # [16, L/16]
        return np.tile(w, (8, 1))  # [128, L/16]

    idx_lo = np.zeros((n_cores, P, nblk * c_lo * 8), dtype=np.int16)
    idx_hi = np.zeros((n_cores, P, max(1, nblk * c_hi * 8)), dtype=np.int16)
    dl_arr = np.full((n_cores, P, nblk * cpb), 300.0, dtype=np.float32)

    for r in range(n_cores):
        lo_col = 0
        hi_col = 0
        for b0, g in groups:
            lo_ids = []
            hi_ids = []
            for b in range(b0, b0 + g):
                ll = lo_lists[r][b]
                ll = ll + [0] * (c_lo * P - len(ll))
                lo_ids.extend(ll)
                hl = hi_lists[r][b]
                hl = hl + [0] * (c_hi * P - len(hl))
                hi_ids.extend(hl)
                # dst-local ids, chunk-major (lo chunks then hi chunks)
                dl_pad_lo = lo_dl[r][b] + [300] * (c_lo * P - len(lo_dl[r][b]))
                dl_pad_hi = hi_dl[r][b] + [300] * (c_hi * P - len(hi_dl[r][b]))
                dl_all = dl_pad_lo + dl_pad_hi
                for c in range(cpb):
                    dl_arr[r, :, b * cpb + c] = dl_all[c * P : (c + 1) * P]
            w = wrap_idx(lo_ids)
            idx_lo[r][:, lo_col : lo_col + w.shape[1]] = w
            lo_col += w.shape[1]
            if c_hi > 0:
                w = wrap_idx(hi_ids)
                idx_hi[r][:, hi_col : hi_col + w.shape[1]] = w
                hi_col += w.shape[1]

    dinv_own = np.zeros((n_cores, P, nblk), dtype=np.float32)
    for r in range(n_cores):
        own = dinv[r * npc : (r + 1) * npc]
        own = np.pad(own, (0, npc_pad - npc))
        dinv_own[r] = own.reshape(nblk, P).T

    return dict(
        n_cores=n_cores,
        N=N,
        E=E,
        nblkA=nblkA,
        rA=rA,
        rB=rB,
        npc=npc,
        nblk=nblk,
        npc_pad=npc_pad,
        c_lo=c_lo,
        c_hi=c_hi,
        groups=groups,
        idx_lo=idx_lo,
        idx_hi=idx_hi,
        dl=dl_arr,
        dinv_own=dinv_own,
    )


# ----------------------------------------------------------------------------
# Kernel builder (same BIR for all cores; per-core data via input tensors)
# ----------------------------------------------------------------------------
def build_kernel(plan, DIN, F1, F2, F3, skip=(), repeat=1, gbufs=3,
                 sbufs=4, tbufs=3):
    n_cores = plan["n_cores"]
    N = plan["N"]
    nblkA = plan["nblkA"]
    rA = plan["rA"]
    rB = plan["rB"]
    nblk = plan["nblk"]
    npc = plan["npc"]
    npc_pad = plan["npc_pad"]
    c_lo = plan["c_lo"]
    c_hi = plan["c_hi"]
    cpb = c_lo + c_hi
    groups = plan["groups"]
    rtA = n_cores * rA
    rtB = n_cores * rB
    rg = [list(range(n_cores))]

    nc = bacc.Bacc("TRN2", target_bir_lowering=False, debug=False,
                   num_devices=n_cores, num_swdge_queues=4)
    import itertools
    _gq = itertools.count()

    # ---- I/O ----
    aT0 = nc.dram_tensor("aT0", [P, npc_pad], BF16, kind="ExternalInput")
    W1 = nc.dram_tensor("W1", [P, F1], BF16, kind="ExternalInput")
    W2 = nc.dram_tensor("W2", [P, F2], BF16, kind="ExternalInput")
    W3 = nc.dram_tensor("W3", [P, F3], BF16, kind="ExternalInput")
    g1 = nc.dram_tensor("g1", [P, 1], F32, kind="ExternalInput")
    be1 = nc.dram_tensor("be1", [P, 1], F32, kind="ExternalInput")
    g2 = nc.dram_tensor("g2", [P, 1], F32, kind="ExternalInput")
    be2 = nc.dram_tensor("be2", [P, 1], F32, kind="ExternalInput")
    b3b = nc.dram_tensor("b3b", [P, F3], F32, kind="ExternalInput")
    iota_in = nc.dram_tensor("iota", [P, P], BF16, kind="ExternalInput")
    ident_in = nc.dram_tensor("ident", [P, P], BF16, kind="ExternalInput")
    idx_lo_in = nc.dram_tensor("idx_lo", list(plan["idx_lo"].shape[1:]), I16,
                               kind="ExternalInput")
    idx_hi_in = nc.dram_tensor("idx_hi", list(plan["idx_hi"].shape[1:]), I16,
                               kind="ExternalInput")
    dl_in = nc.dram_tensor("dl", [P, nblk * cpb], BF16, kind="ExternalInput")
    dinv_in = nc.dram_tensor("dinv_own", [P, nblk], F32, kind="ExternalInput")
    out_t = nc.dram_tensor("out", [npc, F3], F32, kind="ExternalOutput")

    with tile.TileContext(nc) as tc, ExitStack() as ctx:
        nc.gpsimd.load_library(library_config.mlp)

        sb = ctx.enter_context(tc.tile_pool(name="sb", bufs=1))
        # persistent sbuf state
        aT_a = sb.tile([P, npc_pad], BF16, tag="aT_a")
        aT_b = sb.tile([P, npc_pad], BF16, tag="aT_b")
        u_own = sb.tile([P, nblk, max(F1, F2)], BF16, tag="u_own")
        u_own3 = sb.tile([P, nblk, P], BF16, tag="u_own3")
        z_own = sb.tile([P, nblk, max(F1, F2)], BF16, tag="z_own")
        w_sb = sb.tile([P, F1 + F2 + F3], BF16, tag="w_sb")
        iota_t = sb.tile([P, P], BF16, tag="iota_t")
        ident_t = sb.tile([P, P], BF16, tag="ident_t")
        ones_t = sb.tile([P, 1], BF16, tag="ones_t")
        dinv_t = sb.tile([P, nblk], F32, tag="dinv_t")
        dl_t = sb.tile([P, nblk * cpb], BF16, tag="dl_t")
        ilo_t = sb.tile(list(plan["idx_lo"].shape[1:]), I16, tag="ilo_t")
        ihi_t = sb.tile(list(plan["idx_hi"].shape[1:]), I16, tag="ihi_t")
        bnp_t = sb.tile([P, 4], F32, tag="bnp_t")  # g1 be1 g2 be2
        b3_t = sb.tile([P, F3], F32, tag="b3_t")

        nc.sync.dma_start(w_sb[:, 0:F1], W1[:])
        nc.sync.dma_start(w_sb[:, F1:F1 + F2], W2[:])
        nc.sync.dma_start(w_sb[:, F1 + F2:], W3[:])
        nc.sync.dma_start(iota_t[:], iota_in[:])
        nc.sync.dma_start(ident_t[:], ident_in[:])
        nc.sync.dma_start(dinv_t[:], dinv_in[:])
        nc.sync.dma_start(dl_t[:], dl_in[:])
        nc.sync.dma_start(ilo_t[:], idx_lo_in[:])
        if c_hi > 0:
            nc.sync.dma_start(ihi_t[:], idx_hi_in[:])
        nc.sync.dma_start(bnp_t[:, 0:1], g1[:])
        nc.sync.dma_start(bnp_t[:, 1:2], be1[:])
        nc.sync.dma_start(bnp_t[:, 2:3], g2[:])
        nc.sync.dma_start(bnp_t[:, 3:4], be2[:])
        nc.sync.dma_start(b3_t[:], b3b[:])
        nc.sync.dma_start(aT_a[:], aT0[:])
        nc.gpsimd.memset(ones_t[:], 1.0)
        nc.gpsimd.memset(u_own3[:, :, F3:], 0.0)

        # DRAM scratch
        dram = ctx.enter_context(tc.tile_pool(name="dram", bufs=1,
                                              space="DRAM"))
        u1A = dram.tile([rA, F1], BF16, tag="u1A")
        u1B = dram.tile([rB, F1], BF16, tag="u1B")
        u2A = dram.tile([rA, F2], BF16, tag="u2A")
        u2B = dram.tile([rB, F2], BF16, tag="u2B")
        u3A = dram.tile([rA, P], BF16, tag="u3A")
        u3B = dram.tile([rB, P], BF16, tag="u3B")
        st_in1 = dram.tile([P, 2], F32, tag="st_in1")
        st_in2 = dram.tile([P, 2], F32, tag="st_in2")

        # working pools
        psum_mm = ctx.enter_context(
            tc.tile_pool(name="psum_mm", bufs=2, space="PSUM"))
        psum_agg = ctx.enter_context(
            tc.tile_pool(name="psum_agg", bufs=2, space="PSUM"))
        psum_st = ctx.enter_context(
            tc.tile_pool(name="psum_st", bufs=2, space="PSUM"))
        spool = ctx.enter_context(tc.tile_pool(name="spool", bufs=sbufs))
        gpool = ctx.enter_context(tc.tile_pool(name="gpool", bufs=gbufs))
        tpool = ctx.enter_context(tc.tile_pool(name="tpool", bufs=tbufs))

        gbmax = max(g for _, g in groups)

        def layer(l, aT_in, aT_out, F_in, F_out, w_off, udA, udB, ufA, ufB,
                  is_last, g_col=None, be_col=None, st_in=None, st_out=None):
            # ---------------- Phase A: dense matmul + u table ----------
            uo = u_own3 if is_last else u_own
            wtab = P if is_last else F_out

            def emit_half(ud, uf, b0, nb):
                nc.sync.dma_start(
                    ud[:].rearrange("(b p) f -> p b f", p=P),
                    uo[:, b0:b0 + nb, :wtab],
                )
                if n_cores > 1 and "coll" not in skip:
                    nc.gpsimd.collective_compute(
                        "AllGather", ALU.bypass, replica_groups=rg,
                        ins=[ud[:].opt()], outs=[uf[:].opt()],
                    )
                else:
                    nc.sync.dma_start(uf[0:nb * P, :], ud[:])

            for b in range(nblk):
                h_ps = psum_mm.tile([P, F_out], F32, tag="mm")
                nc.tensor.matmul(
                    h_ps[:],
                    lhsT=aT_in[:, b * P:(b + 1) * P],
                    rhs=w_sb[:, w_off:w_off + F_out],
                    start=True, stop=True,
                )
                nc.scalar.activation(uo[:, b, :F_out], h_ps[:], AF.Copy,
                                     scale=dinv_t[:, b:b + 1])
                if b == nblkA - 1:
                    emit_half(udA, ufA, 0, nblkA)
            emit_half(udB, ufB, nblkA, nblk - nblkA)

            # ---------------- Phase B: gather + segment matmul ---------
            if not is_last:
                st_s = psum_st.tile([P, 1], F32, tag="st_s")
                st_q = psum_st.tile([P, 1], F32, tag="st_q")
            lo_col = 0
            hi_col = 0
            for b0, g in groups:
                n_lo = g * c_lo * P
                lo_t = gpool.tile([P, gbmax * c_lo, P], BF16, tag="lo")
                if "noload" in skip:
                    pass
                elif "seqload" in skip:
                    nc.gpsimd.dma_start(
                        lo_t[:, :g * c_lo, :],
                        ufA[0:n_lo, :].rearrange(
                            "(c p) f -> p c f", p=P),
                    )
                else:
                    nc.gpsimd.dma_gather(
                        lo_t[:, :g * c_lo, :], ufA[:],
                        ilo_t[:, lo_col:lo_col + n_lo // 16],
                        n_lo, n_lo, P, single_packet=False,
                        queue_num=next(_gq) % 4,
                    )
                lo_col += n_lo // 16
                if c_hi > 0:
                    n_hi = g * c_hi * P
                    hi_t = gpool.tile([P, gbmax * c_hi, P], BF16, tag="hi")
                    if "noload" in skip:
                        pass
                    elif "seqload" in skip:
                        nc.gpsimd.dma_start(
                            hi_t[:, :g * c_hi, :],
                            ufB[0:n_hi, :].rearrange(
                                "(c p) f -> p c f", p=P),
                        )
                    else:
                        nc.gpsimd.dma_gather(
                            hi_t[:, :g * c_hi, :],
                            ufB[:],
                            ihi_t[:, hi_col:hi_col + n_hi // 16],
                            n_hi, n_hi, P, single_packet=False,
                            queue_num=next(_gq) % 4,
                        )
                    hi_col += n_hi // 16
                for bb in range(g):
                    b = b0 + bb
                    agg = psum_agg.tile([P, F_out], F32, tag="agg")
                    if "seg" in skip:
                        nc.vector.memset(agg[:], 0.0)
                    else:
                        s_w = spool.tile([P, cpb, P], BF16, tag="s")
                        nc.vector.tensor_tensor(
                            out=s_w[:],
                            in0=iota_t[:, None, :].to_broadcast([P, cpb, P]),
                            in1=dl_t[:, b * cpb:(b + 1) * cpb].to_broadcast(
                                [P, cpb, P]),
                            op=ALU.is_equal,
                        )
                        for c in range(cpb):
                            if c < c_lo:
                                rhs = lo_t[:, bb * c_lo + c, :F_out]
                            else:
                                rhs = hi_t[:, bb * c_hi + (c - c_lo), :F_out]
                            nc.tensor.matmul(
                                agg[:], lhsT=s_w[:, c, :], rhs=rhs,
                                start=(c == 0), stop=(c == cpb - 1),
                            )
                    # epilogue: z = dinv * (agg + u_own)
                    uo = u_own3 if is_last else u_own
                    t_t = tpool.tile([P, F_out], F32, tag="t")
                    nc.vector.tensor_tensor(
                        out=t_t[:], in0=agg[:], in1=uo[:, b, :F_out],
                        op=ALU.add,
                    )
                    if is_last:
                        z3 = tpool.tile([P, F_out], F32, tag="z3")
                        nc.scalar.activation(z3[:], t_t[:], AF.Copy,
                                             scale=dinv_t[:, b:b + 1])
                        o_t = tpool.tile([P, F_out], F32, tag="o")
                        nc.vector.tensor_tensor(out=o_t[:], in0=z3[:],
                                                in1=b3_t[:], op=ALU.add)
                        hi_row = min(npc, (b + 1) * P) - b * P
                        nc.sync.dma_start(out_t[b * P:b * P + hi_row, :],
                                          o_t[:hi_row, :])
                    else:
                        nc.scalar.activation(z_own[:, b, :F_out], t_t[:],
                                             AF.Copy,
                                             scale=dinv_t[:, b:b + 1])
                        z2 = tpool.tile([P, F_out], BF16, tag="z2")
                        nc.scalar.activation(z2[:], z_own[:, b, :F_out],
                                             AF.Square)
                        nc.tensor.matmul(st_s[:], lhsT=z_own[:, b, :F_out],
                                         rhs=ones_t[:],
                                         start=(b == 0), stop=(b == nblk - 1),
                                         skip_group_check=True)
                        nc.tensor.matmul(st_q[:], lhsT=z2[:], rhs=ones_t[:],
                                         start=(b == 0), stop=(b == nblk - 1),
                                         skip_group_check=True)
            if is_last:
                return

            # ---------------- Phase C: BN stats allreduce + coeffs -----
            st_sb = tpool.tile([P, 2], F32, tag="stsb")
            nc.vector.tensor_copy(st_sb[:, 0:1], st_s[:])
            nc.vector.tensor_copy(st_sb[:, 1:2], st_q[:])
            nc.sync.dma_start(st_in[:], st_sb[:])
            if n_cores > 1 and "coll" not in skip:
                nc.gpsimd.collective_compute(
                    "AllReduce", ALU.add, replica_groups=rg,
                    ins=[st_in[:].opt()], outs=[st_out[:].opt()],
                )
            else:
                nc.sync.dma_start(st_out[:], st_in[:])
            st_g = tpool.tile([P, 2], F32, tag="stg")
            nc.sync.dma_start(st_g[:], st_out[:])
            m_t = tpool.tile([P, 1], F32, tag="m")
            nc.scalar.activation(m_t[:], st_g[:, 0:1], AF.Copy, scale=1.0 / N)
            q_t = tpool.tile([P, 1], F32, tag="q")
            nc.scalar.activation(q_t[:], st_g[:, 1:2], AF.Copy, scale=1.0 / N)
            m2_t = tpool.tile([P, 1], F32, tag="m2")
            nc.scalar.activation(m2_t[:], m_t[:], AF.Square)
            v_t = tpool.tile([P, 1], F32, tag="v")
            nc.vector.tensor_tensor(out=v_t[:], in0=q_t[:], in1=m2_t[:],
                                    op=ALU.subtract)
            ve_t = tpool.tile([P, 1], F32, tag="ve")
            nc.vector.tensor_scalar(out=ve_t[:], in0=v_t[:], scalar1=1e-5,
                                    scalar2=None, op0=ALU.add)
            sd_t = tpool.tile([P, 1], F32, tag="sd")
            nc.scalar.activation(sd_t[:], ve_t[:], AF.Sqrt)
            inv_t = tpool.tile([P, 1], F32, tag="inv")
            nc.vector.reciprocal(inv_t[:], sd_t[:])
            a_t = tpool.tile([P, 1], F32, tag="A")
            nc.vector.tensor_tensor(out=a_t[:], in0=bnp_t[:, g_col:g_col + 1],
                                    in1=inv_t[:], op=ALU.mult)
            ma_t = tpool.tile([P, 1], F32, tag="mA")
            nc.vector.tensor_tensor(out=ma_t[:], in0=m_t[:], in1=a_t[:],
                                    op=ALU.mult)
            bb_t = tpool.tile([P, 1], F32, tag="B")
            nc.vector.tensor_tensor(out=bb_t[:],
                                    in0=bnp_t[:, be_col:be_col + 1],
                                    in1=ma_t[:], op=ALU.subtract)

            # ---------------- Phase D: transpose + BN apply + relu -----
            for b in range(nblk):
                zT = psum_mm.tile([P, P], BF16, tag="mm")
                nc.tensor.transpose(zT[:], z_own[:, b, :F_out], ident_t[:])
                nc.scalar.activation(aT_out[:, b * P:(b + 1) * P], zT[:],
                                     AF.Relu, bias=bb_t[:], scale=a_t[:])

        for _rep in range(repeat):
            uf1A = dram.tile([rtA, F1], BF16, tag=f"uf1A_{_rep}",
                             addr_space="Shared")
            uf1B = dram.tile([rtB, F1], BF16, tag=f"uf1B_{_rep}",
                             addr_space="Shared")
            uf2A = dram.tile([rtA, F2], BF16, tag=f"uf2A_{_rep}",
                             addr_space="Shared")
            uf2B = dram.tile([rtB, F2], BF16, tag=f"uf2B_{_rep}",
                             addr_space="Shared")
            uf3A = dram.tile([rtA, P], BF16, tag=f"uf3A_{_rep}",
                             addr_space="Shared")
            uf3B = dram.tile([rtB, P], BF16, tag=f"uf3B_{_rep}",
                             addr_space="Shared")
            st_out1 = dram.tile([P, 2], F32, tag=f"st_out1_{_rep}",
                                addr_space="Shared")
            st_out2 = dram.tile([P, 2], F32, tag=f"st_out2_{_rep}",
                                addr_space="Shared")
            if _rep > 0:
                nc.sync.dma_start(aT_a[:], aT0[:])
            layer(1, aT_a, aT_b, DIN, F1, 0, u1A, u1B, uf1A, uf1B, False,
                  0, 1, st_in1, st_out1)
            layer(2, aT_b, aT_a, F1, F2, F1, u2A, u2B, uf2A, uf2B, False,
                  2, 3, st_in2, st_out2)
            layer(3, aT_a, None, F2, F3, F1 + F2, u3A, u3B, uf3A, uf3B,
                  True)

    nc.compile()
    return nc


# ----------------------------------------------------------------------------
# Host entry point
# ----------------------------------------------------------------------------
def make_in_maps(plan, inputs, DIN, F1, F2, F3):
    n_cores = plan["n_cores"]
    npc = plan["npc"]
    npc_pad = plan["npc_pad"]
    x = np.asarray(inputs["x"], dtype=np.float32)
    import ml_dtypes
    bf16 = ml_dtypes.bfloat16
    iota = np.tile(np.arange(P)[None, :], (P, 1)).astype(bf16)
    ident = np.eye(P, dtype=bf16)
    b3b = np.tile(np.asarray(inputs["b3"], np.float32)[None, :], (P, 1))
    col = lambda v: np.asarray(v, np.float32).reshape(P, 1)
    in_maps = []
    for r in range(n_cores):
        xr = x[r * npc:(r + 1) * npc]
        aT0 = np.zeros((P, npc_pad), bf16)
        aT0[:, :npc] = xr.T.astype(bf16)
        in_maps.append({
            "aT0": aT0,
            "W1": np.asarray(inputs["W1"], np.float32).astype(bf16),
            "W2": np.asarray(inputs["W2"], np.float32).astype(bf16),
            "W3": np.asarray(inputs["W3"], np.float32).astype(bf16),
            "g1": col(inputs["g1"]), "be1": col(inputs["be1"]),
            "g2": col(inputs["g2"]), "be2": col(inputs["be2"]),
            "b3b": b3b, "iota": iota, "ident": ident,
            "idx_lo": plan["idx_lo"][r], "idx_hi": plan["idx_hi"][r],
            "dl": plan["dl"][r].astype(ml_dtypes.bfloat16), "dinv_own": plan["dinv_own"][r],
        })
    return in_maps


_CACHE = {}


def kernel(**inputs):
    x = np.asarray(inputs["x"], dtype=np.float32)
    N, DIN = x.shape
    F1 = inputs["W1"].shape[1]
    F2 = inputs["W2"].shape[1]
    F3 = inputs["W3"].shape[1]
    edge_index = np.asarray(inputs["edge_index"])

    key = (N, DIN, F1, F2, F3, hash(edge_index.tobytes()))
    if key not in _CACHE:
        plan = make_plan(edge_index, N)
        nc = build_kernel(plan, DIN, F1, F2, F3)
        _CACHE[key] = (plan, nc)
    plan, nc = _CACHE[key]

    in_maps = make_in_maps(plan, inputs, DIN, F1, F2, F3)
    res = run_bass_kernel_spmd(nc, in_maps, core_ids=list(range(plan["n_cores"])))
    out = np.concatenate([res.results[r]["out"] for r in range(plan["n_cores"])],
                         axis=0)
    return out.astype(np.float32)


if __name__ == "__main__":
    import reference

    inputs = {k: np.asarray(v) for k, v in reference.setup_inputs().items()}
    out = kernel(**inputs)
    exp = np.asarray(reference.reference(**inputs))
    err = np.abs(out - exp).max() / (np.abs(exp).max() + 1e-30)
    print("Relative error:", err)

